# revision 3
# baseline (speedup 1.0000x reference)
"""InternLM2 attention layer on 8 Trainium2 NeuronCores.

Tensor-parallel over kv-head groups: core c gets wqkv rows [768c, 768c+768)
(4 q heads + k + v of kv-group c) and wo columns [512c, 512c+512).
Each core computes its 4 heads' attention and a partial output projection;
the host sums the 8 partials (the TP all-reduce) as the unshard step.

Device layout choices:
  - QKV computed transposed ([feat, seq]) so Q.K^T needs no transposes.
  - Scores computed pre-transposed S^T[sk, sq] so the PV matmul consumes
    exp(S^T) directly; softmax denominator via an all-ones matmul.
  - float32r matmuls (fp32 bits, ~tf32 precision, 4x fp32 throughput).
  - Causal mask baked in: fully-masked score tiles are skipped, diagonal
    tiles multiplied by a 0/1 mask after exp.
"""
import os
import math
import numpy as np

import concourse.bacc as bacc
import concourse.tile as tile
from concourse import mybir
from concourse.bass_utils import run_bass_kernel_spmd

S = 2048          # sequence length
H = 4096          # hidden size
HD = 128          # head dim
G = 4             # q heads per kv head (= per core)
FEAT = (G + 2) * HD   # 768 wqkv rows per core
NCORES = 8
CH = 512          # seq chunk (matmul moving-dim)
NCH = S // CH     # 4
KT = H // 128     # 32 contraction tiles for the projections
NSEQT = S // 128  # 16
ROPE_BASE = 1000000.0

F32 = mybir.dt.float32
F32R = mybir.dt.float32r

_EXEC_TIME_NS = None


def build_nc():
    nc = bacc.Bacc("TRN2", target_bir_lowering=False, debug=False)

    xT = nc.declare_dram_parameter("xT", [H, S], F32, isOutput=False)
    wqkvT = nc.declare_dram_parameter("wqkvT", [H, FEAT], F32, isOutput=False)
    woT = nc.declare_dram_parameter("woT", [G * HD, H], F32, isOutput=False)
    cosT = nc.declare_dram_parameter("cosT", [HD, S], F32, isOutput=False)
    sinT = nc.declare_dram_parameter("sinT", [HD, S], F32, isOutput=False)
    maskd = nc.declare_dram_parameter("maskd", [G, 128, CH], F32, isOutput=False)
    ident = nc.declare_dram_parameter("ident", [128, 128], F32, isOutput=False)
    ones = nc.declare_dram_parameter("ones", [128, 1], F32, isOutput=False)
    out = nc.declare_dram_parameter("out", [S, H], F32, isOutput=True)

    ExpF = mybir.ActivationFunctionType.Exp

    with tile.TileContext(nc) as tc:
        with tc.tile_pool(name="pers", bufs=1) as pers:
            # persistent across phases
            qT = [pers.tile([128, S], F32R, tag=f"qT{h}", name=f"qT{h}")
                  for h in range(G)]
            kT_sb = pers.tile([128, S], F32R, tag="kT")
            v_sb = pers.tile([128, NSEQT, HD], F32R, tag="v")  # [sk, t, hd]
            ident_sb = pers.tile([128, 128], F32, tag="ident")
            ones_sb = pers.tile([128, 1], F32R, tag="ones")
            nc.sync.dma_start(out=ident_sb, in_=ident[:])
            nc.sync.dma_start(out=ones_sb, in_=ones[:].bitcast(F32R))

            # ---------------- Phase 1: QKV projection + RoPE ----------------
            with tc.tile_pool(name="p1", bufs=1) as p1, \
                 tc.tile_pool(name="xt", bufs=4) as xpool, \
                 tc.tile_pool(name="rope", bufs=3) as rpool, \
                 tc.tile_pool(name="psq", bufs=1, space="PSUM") as psq, \
                 tc.tile_pool(name="psvt", bufs=2, space="PSUM") as psvt:
                w_sb = p1.tile([128, KT, FEAT], F32R, tag="w")
                wqkvT_r = wqkvT[:].bitcast(F32R).rearrange(
                    "(kt p) f -> kt p f", p=128)
                for k in range(KT):
                    nc.sync.dma_start(out=w_sb[:, k, :], in_=wqkvT_r[k])
                cos_sb = p1.tile([128, S], F32, tag="cos")
                sin_sb = p1.tile([128, S], F32, tag="sin")
                nc.sync.dma_start(out=cos_sb, in_=cosT[:])
                nc.sync.dma_start(out=sin_sb, in_=sinT[:])
                vT_sb = p1.tile([128, S], F32, tag="vT")  # v as [hd, seq]

                xT_r = xT[:].bitcast(F32R)
                for ch in range(NCH):
                    cc = slice(ch * CH, (ch + 1) * CH)
                    ps = [psq.tile([128, CH], F32, tag=f"ps{m}", name=f"ps{m}")
                          for m in range(6)]
                    for k in range(KT):
                        xt = xpool.tile([128, CH], F32R, tag="x")
                        nc.sync.dma_start(
                            out=xt, in_=xT_r[k * 128:(k + 1) * 128, cc])
                        for m in range(6):
                            nc.tensor.matmul(
                                ps[m],
                                lhsT=w_sb[:, k, m * 128:(m + 1) * 128],
                                rhs=xt,
                                start=(k == 0),
                                stop=(k == KT - 1),
                            )
                    # drain: RoPE for q heads (m=0..3) and k (m=4)
                    for m in range(5):
                        dst = qT[m] if m < G else kT_sb
                        qp = rpool.tile([128, CH], F32, tag="qp")
                        nc.scalar.copy(qp, ps[m])
                        sh = rpool.tile([128, CH], F32, tag="sh")
                        nc.sync.dma_start(out=sh[0:64, :], in_=qp[64:128, :])
                        nc.sync.dma_start(out=sh[64:128, :], in_=qp[0:64, :])
                        nc.vector.tensor_mul(dst[:, cc], qp, cos_sb[:, cc])
                        nc.vector.tensor_mul(sh, sh, sin_sb[:, cc])
                        nc.vector.tensor_add(dst[:, cc], dst[:, cc], sh)
                    # v: plain copy (no rope)
                    nc.scalar.copy(vT_sb[:, cc], ps[5])

                # v transposes: [hd, seq] -> per-tile [sk, hd]
                for t in range(NSEQT):
                    pst = psvt.tile([128, 128], F32, tag="vt")
                    nc.tensor.transpose(
                        pst, vT_sb[:, t * 128:(t + 1) * 128], ident_sb)
                    nc.any.tensor_copy(v_sb[:, t, :], pst)

            # ---------------- Phase 2+3: attention + out-proj ----------------
            with tc.tile_pool(name="p2", bufs=1) as p2, \
                 tc.tile_pool(name="epool", bufs=6) as epool, \
                 tc.tile_pool(name="spool", bufs=4) as spool, \
                 tc.tile_pool(name="opool", bufs=3) as opool, \
                 tc.tile_pool(name="psst", bufs=2, space="PSUM") as psst, \
                 tc.tile_pool(name="pspv", bufs=2, space="PSUM") as pspv, \
                 tc.tile_pool(name="psden", bufs=2, space="PSUM") as psden, \
                 tc.tile_pool(name="pso", bufs=2, space="PSUM") as pso:
                wo_sb = p2.tile([128, G, H], F32R, tag="wo")
                woT_r = woT[:].bitcast(F32R).rearrange("(g p) o -> g p o", p=128)
                for g in range(G):
                    nc.sync.dma_start(out=wo_sb[:, g, :], in_=woT_r[g])
                masks = p2.tile([128, G, CH], F32, tag="masks")
                nc.sync.dma_start(
                    out=masks, in_=maskd[:].rearrange("r p c -> p r c"))
                attnT = [p2.tile([128, S], F32R, tag=f"attnT{h}", name=f"attnT{h}")
                         for h in range(G)]

                for h in range(G):
                    for ch in range(NCH):
                        cc = slice(ch * CH, (ch + 1) * CH)
                        nsk = G * ch + G
                        pv = pspv.tile([128, CH], F32, tag="pv")
                        den = psden.tile([1, CH], F32, tag="den")
                        for t in range(nsk):
                            st = psst.tile([128, CH], F32, tag="st")
                            nc.tensor.matmul(
                                st,
                                lhsT=kT_sb[:, t * 128:(t + 1) * 128],
                                rhs=qT[h][:, cc],
                                start=True, stop=True)
                            E = epool.tile([128, CH], F32R, tag="E")
                            nc.scalar.activation(E, st, ExpF)
                            if t >= G * ch:  # diagonal tile: causal 0/1 mask
                                nc.vector.tensor_mul(
                                    E, E, masks[:, t - G * ch, :])
                            nc.tensor.matmul(
                                pv, lhsT=v_sb[:, t, :], rhs=E,
                                start=(t == 0), stop=(t == nsk - 1))
                            nc.tensor.matmul(
                                den, lhsT=ones_sb, rhs=E,
                                start=(t == 0), stop=(t == nsk - 1))
                        recip = spool.tile([1, CH], F32, tag="recip")
                        nc.vector.reciprocal(recip, den[0:1, :])
                        bc = spool.tile([128, CH], F32, tag="bc")
                        nc.gpsimd.partition_broadcast(bc, recip[0:1, :])
                        nc.vector.tensor_mul(attnT[h][:, cc], pv, bc)

                # out projection: out[sq, :] partial
                for mt in range(NSEQT):
                    sq = slice(mt * 128, (mt + 1) * 128)
                    for n in range(H // CH):
                        po = pso.tile([128, CH], F32, tag="po")
                        for g in range(G):
                            nc.tensor.matmul(
                                po,
                                lhsT=attnT[g][:, sq],
                                rhs=wo_sb[:, g, n * CH:(n + 1) * CH],
                                start=(g == 0), stop=(g == G - 1))
                        ot = opool.tile([128, CH], F32, tag="ot")
                        nc.any.tensor_copy(ot, po)
                        nc.sync.dma_start(out=out[:][sq, n * CH:(n + 1) * CH],
                                          in_=ot)

    nc.finalize()
    return nc


def _host_inputs(hidden_states, wqkv, wo, attention_mask, position_ids):
    x = np.asarray(hidden_states, dtype=np.float32)[0]          # [S, H]
    wqkv = np.asarray(wqkv, dtype=np.float32)                   # [6144, H]
    wo = np.asarray(wo, dtype=np.float32)                       # [H, H]
    pos = np.asarray(position_ids)[0].astype(np.int64)          # [S]

    xT = np.ascontiguousarray(x.T)                              # [H, S]

    inv = 1.0 / (ROPE_BASE ** (np.arange(0, HD, 2, dtype=np.float32) / HD))
    freqs = np.outer(pos.astype(np.float32), inv)               # [S, 64]
    emb = np.concatenate([freqs, freqs], axis=-1)               # [S, 128]
    cosT = np.ascontiguousarray(np.cos(emb).T.astype(np.float32))   # [128, S]
    sinT = np.sin(emb).T.astype(np.float32)
    sinT_signed = sinT.copy()
    sinT_signed[:64] = -sinT_signed[:64]
    sinT_signed = np.ascontiguousarray(sinT_signed)

    # diagonal-tile causal 0/1 masks, [G, 128, CH]
    r_off = (np.arange(G) * 128)[:, None, None]
    i_idx = np.arange(128)[None, :, None]
    c_idx = np.arange(CH)[None, None, :]
    maskd = (c_idx >= r_off + i_idx).astype(np.float32)

    ident = np.eye(128, dtype=np.float32)
    ones = np.ones((128, 1), dtype=np.float32)

    scale = np.float32(1.0 / math.sqrt(HD))
    in_maps = []
    for c in range(NCORES):
        wq_c = wqkv[FEAT * c:FEAT * (c + 1)].copy()             # [768, H]
        wq_c[:G * HD] *= scale                                  # fold softmax scale
        wqkvT_c = np.ascontiguousarray(wq_c.T)                  # [H, 768]
        woT_c = np.ascontiguousarray(wo[:, G * HD * c:G * HD * (c + 1)].T)
        in_maps.append({
            "xT": xT,
            "wqkvT": wqkvT_c,
            "woT": woT_c,
            "cosT": cosT,
            "sinT": sinT_signed,
            "maskd": maskd,
            "ident": ident,
            "ones": ones,
        })
    return in_maps


def kernel(hidden_states, wqkv, wo, attention_mask, position_ids):
    global _EXEC_TIME_NS
    in_maps = _host_inputs(hidden_states, wqkv, wo, attention_mask,
                           position_ids)
    nc = build_nc()
    trace = os.environ.get("TRN_KERNEL_TRACE", "0") == "1"
    res = run_bass_kernel_spmd(nc, in_maps, core_ids=list(range(NCORES)),
                               trace=trace)
    _EXEC_TIME_NS = res.exec_time_ns
    out = np.zeros((S, H), dtype=np.float32)
    for c in range(NCORES):
        out += res.results[c]["out"]
    return out.reshape(1, S, H).astype(np.float32)


# revision 6
# speedup vs baseline: 1.0541x; 1.0541x over previous
"""InternLM2 attention layer on 8 Trainium2 NeuronCores.

Tensor-parallel over kv-head groups: core c gets wqkv rows [768c, 768c+768)
(4 q heads + k + v of kv-group c) and wo columns [512c, 512c+512).
Each core computes its 4 heads' attention and a partial output projection;
the host sums the 8 partials (the TP all-reduce) as the unshard step.

Device layout choices:
  - QKV computed transposed ([feat, seq]) so Q.K^T needs no transposes.
  - Scores computed pre-transposed S^T[sk, sq] so the PV matmul consumes
    exp(S^T) directly; softmax denominator via an all-ones matmul.
  - float32r matmuls (fp32 bits, ~tf32 precision, 4x fp32 throughput).
  - Causal mask baked in: fully-masked score tiles are skipped, diagonal
    tiles multiplied by a 0/1 mask after exp.
"""
import os
import math
import numpy as np

import concourse.bacc as bacc
import concourse.tile as tile
from concourse import mybir
from concourse.bass_utils import run_bass_kernel_spmd

S = 2048          # sequence length
H = 4096          # hidden size
HD = 128          # head dim
G = 4             # q heads per kv head (= per core)
FEAT = (G + 2) * HD   # 768 wqkv rows per core
NCORES = 8
CH = 512          # seq chunk (matmul moving-dim)
NCH = S // CH     # 4
KT = H // 128     # 32 contraction tiles for the projections
NSEQT = S // 128  # 16
ROPE_BASE = 1000000.0

F32 = mybir.dt.float32
F32R = mybir.dt.float32r

_EXEC_TIME_NS = None


def build_nc():
    nc = bacc.Bacc("TRN2", target_bir_lowering=False, debug=False)

    xT = nc.declare_dram_parameter("xT", [H, S], F32, isOutput=False)
    wqkvT = nc.declare_dram_parameter("wqkvT", [H, FEAT], F32, isOutput=False)
    woT = nc.declare_dram_parameter("woT", [G * HD, H], F32, isOutput=False)
    cosT = nc.declare_dram_parameter("cosT", [HD, S], F32, isOutput=False)
    sinT = nc.declare_dram_parameter("sinT", [HD, S], F32, isOutput=False)
    maskd = nc.declare_dram_parameter("maskd", [G, 128, CH], F32, isOutput=False)
    ident = nc.declare_dram_parameter("ident", [128, 128], F32, isOutput=False)
    ones = nc.declare_dram_parameter("ones", [128, 1], F32, isOutput=False)
    out = nc.declare_dram_parameter("out", [S, H], F32, isOutput=True)

    ExpF = mybir.ActivationFunctionType.Exp

    with tile.TileContext(nc) as tc:
        with tc.tile_pool(name="pers", bufs=1) as pers:
            # persistent across phases
            qT = [pers.tile([128, S], F32R, tag=f"qT{h}", name=f"qT{h}")
                  for h in range(G)]
            kT_sb = pers.tile([128, S], F32R, tag="kT")
            v_sb = pers.tile([128, NSEQT, HD], F32R, tag="v")  # [sk, t, hd]
            ident_sb = pers.tile([128, 128], F32, tag="ident")
            ones_sb = pers.tile([128, 1], F32R, tag="ones")
            nc.sync.dma_start(out=ident_sb, in_=ident[:])
            nc.sync.dma_start(out=ones_sb, in_=ones[:].bitcast(F32R))

            # ---------------- Phase 1: QKV projection + RoPE ----------------
            with tc.tile_pool(name="p1", bufs=1) as p1, \
                 tc.tile_pool(name="xt", bufs=6) as xpool, \
                 tc.tile_pool(name="rope", bufs=3) as rpool, \
                 tc.tile_pool(name="psq", bufs=1, space="PSUM") as psq, \
                 tc.tile_pool(name="psvt", bufs=2, space="PSUM") as psvt:
                w_sb = p1.tile([128, KT, FEAT], F32R, tag="w")
                wqkvT_r = wqkvT[:].bitcast(F32R).rearrange(
                    "(kt p) f -> kt p f", p=128)
                cos_sb = p1.tile([128, S], F32, tag="cos")
                sin_sb = p1.tile([128, S], F32, tag="sin")
                vT_sb = p1.tile([128, S], F32, tag="vT")  # v as [hd, seq]

                xT_r = xT[:].bitcast(F32R)
                for ch in range(NCH):
                    cc = slice(ch * CH, (ch + 1) * CH)
                    ps = [psq.tile([128, CH], F32, tag=f"ps{m}", name=f"ps{m}")
                          for m in range(6)]
                    for k in range(KT):
                        if ch == 0:
                            # interleave weight loads with x tiles so the
                            # first matmuls aren't behind the full 12.6MB
                            nc.sync.dma_start(out=w_sb[:, k, :],
                                              in_=wqkvT_r[k])
                        xt = xpool.tile([128, CH], F32R, tag="x")
                        nc.sync.dma_start(
                            out=xt, in_=xT_r[k * 128:(k + 1) * 128, cc])
                        for m in range(6):
                            nc.tensor.matmul(
                                ps[m],
                                lhsT=w_sb[:, k, m * 128:(m + 1) * 128],
                                rhs=xt,
                                start=(k == 0),
                                stop=(k == KT - 1),
                            )
                    if ch == 0:
                        nc.sync.dma_start(out=cos_sb, in_=cosT[:])
                        nc.sync.dma_start(out=sin_sb, in_=sinT[:])
                    # drain: RoPE for q heads (m=0..3) and k (m=4)
                    for m in range(5):
                        dst = qT[m] if m < G else kT_sb
                        qp = rpool.tile([128, CH], F32, tag="qp")
                        if m % 2 == 0:
                            nc.scalar.copy(qp, ps[m])
                        else:
                            nc.vector.tensor_copy(qp, ps[m])
                        sh = rpool.tile([128, CH], F32, tag="sh")
                        nc.sync.dma_start(out=sh[0:64, :], in_=qp[64:128, :])
                        nc.sync.dma_start(out=sh[64:128, :], in_=qp[0:64, :])
                        nc.vector.tensor_mul(dst[:, cc], qp, cos_sb[:, cc])
                        nc.vector.tensor_mul(sh, sh, sin_sb[:, cc])
                        nc.vector.tensor_add(dst[:, cc], dst[:, cc], sh)
                    # v: plain copy (no rope), then transpose this chunk's
                    # v tiles to [sk, hd] while later chunks still project
                    nc.scalar.copy(vT_sb[:, cc], ps[5])
                    for t in range(G * ch, G * ch + G):
                        pst = psvt.tile([128, 128], F32, tag="vt")
                        nc.tensor.transpose(
                            pst, vT_sb[:, t * 128:(t + 1) * 128], ident_sb)
                        nc.any.tensor_copy(v_sb[:, t, :], pst)

            # ---------------- Phase 2+3: attention + out-proj ----------------
            with tc.tile_pool(name="p2", bufs=1) as p2, \
                 tc.tile_pool(name="epool", bufs=8) as epool, \
                 tc.tile_pool(name="spool", bufs=4) as spool, \
                 tc.tile_pool(name="opool", bufs=3) as opool, \
                 tc.tile_pool(name="psst", bufs=2, space="PSUM") as psst, \
                 tc.tile_pool(name="pspv", bufs=2, space="PSUM") as pspv, \
                 tc.tile_pool(name="psden", bufs=2, space="PSUM") as psden, \
                 tc.tile_pool(name="pso", bufs=2, space="PSUM") as pso:
                wo_sb = p2.tile([128, G, H], F32R, tag="wo")
                woT_r = woT[:].bitcast(F32R).rearrange("(g p) o -> g p o", p=128)
                for g in range(G):
                    nc.sync.dma_start(out=wo_sb[:, g, :], in_=woT_r[g])
                masks = p2.tile([128, G, CH], F32, tag="masks")
                nc.sync.dma_start(
                    out=masks, in_=maskd[:].rearrange("r p c -> p r c"))
                attnT = [p2.tile([128, S], F32R, tag=f"attnT{h}", name=f"attnT{h}")
                         for h in range(G)]

                for ch in range(NCH):
                    for h in range(G):
                        cc = slice(ch * CH, (ch + 1) * CH)
                        nsk = G * ch + G
                        pv = pspv.tile([128, CH], F32, tag="pv")
                        den = psden.tile([1, CH], F32, tag="den")
                        for t in range(nsk):
                            st = psst.tile([128, CH], F32, tag="st")
                            nc.tensor.matmul(
                                st,
                                lhsT=kT_sb[:, t * 128:(t + 1) * 128],
                                rhs=qT[h][:, cc],
                                start=True, stop=True)
                            E = epool.tile([128, CH], F32R, tag="E")
                            nc.scalar.activation(E, st, ExpF)
                            if t >= G * ch:  # diagonal tile: causal 0/1 mask
                                nc.vector.tensor_mul(
                                    E, E, masks[:, t - G * ch, :])
                            nc.tensor.matmul(
                                pv, lhsT=v_sb[:, t, :], rhs=E,
                                start=(t == 0), stop=(t == nsk - 1))
                            nc.tensor.matmul(
                                den, lhsT=ones_sb, rhs=E,
                                start=(t == 0), stop=(t == nsk - 1))
                        recip = spool.tile([1, CH], F32, tag="recip")
                        nc.vector.reciprocal(recip, den[0:1, :])
                        bc = spool.tile([128, CH], F32, tag="bc")
                        nc.gpsimd.partition_broadcast(bc, recip[0:1, :])
                        nc.vector.tensor_mul(attnT[h][:, cc], pv, bc)

                # out projection: out[sq, :] partial
                for mt in range(NSEQT):
                    sq = slice(mt * 128, (mt + 1) * 128)
                    for n in range(H // CH):
                        po = pso.tile([128, CH], F32, tag="po")
                        for g in range(G):
                            nc.tensor.matmul(
                                po,
                                lhsT=attnT[g][:, sq],
                                rhs=wo_sb[:, g, n * CH:(n + 1) * CH],
                                start=(g == 0), stop=(g == G - 1))
                        ot = opool.tile([128, CH], F32, tag="ot")
                        nc.vector.tensor_copy(ot, po)
                        nc.sync.dma_start(out=out[:][sq, n * CH:(n + 1) * CH],
                                          in_=ot)

    nc.finalize()
    return nc


def _host_inputs(hidden_states, wqkv, wo, attention_mask, position_ids):
    x = np.asarray(hidden_states, dtype=np.float32)[0]          # [S, H]
    wqkv = np.asarray(wqkv, dtype=np.float32)                   # [6144, H]
    wo = np.asarray(wo, dtype=np.float32)                       # [H, H]
    pos = np.asarray(position_ids)[0].astype(np.int64)          # [S]

    xT = np.ascontiguousarray(x.T)                              # [H, S]

    inv = 1.0 / (ROPE_BASE ** (np.arange(0, HD, 2, dtype=np.float32) / HD))
    freqs = np.outer(pos.astype(np.float32), inv)               # [S, 64]
    emb = np.concatenate([freqs, freqs], axis=-1)               # [S, 128]
    cosT = np.ascontiguousarray(np.cos(emb).T.astype(np.float32))   # [128, S]
    sinT = np.sin(emb).T.astype(np.float32)
    sinT_signed = sinT.copy()
    sinT_signed[:64] = -sinT_signed[:64]
    sinT_signed = np.ascontiguousarray(sinT_signed)

    # diagonal-tile causal 0/1 masks, [G, 128, CH]
    r_off = (np.arange(G) * 128)[:, None, None]
    i_idx = np.arange(128)[None, :, None]
    c_idx = np.arange(CH)[None, None, :]
    maskd = (c_idx >= r_off + i_idx).astype(np.float32)

    ident = np.eye(128, dtype=np.float32)
    ones = np.ones((128, 1), dtype=np.float32)

    scale = np.float32(1.0 / math.sqrt(HD))
    in_maps = []
    for c in range(NCORES):
        wq_c = wqkv[FEAT * c:FEAT * (c + 1)].copy()             # [768, H]
        wq_c[:G * HD] *= scale                                  # fold softmax scale
        wqkvT_c = np.ascontiguousarray(wq_c.T)                  # [H, 768]
        woT_c = np.ascontiguousarray(wo[:, G * HD * c:G * HD * (c + 1)].T)
        in_maps.append({
            "xT": xT,
            "wqkvT": wqkvT_c,
            "woT": woT_c,
            "cosT": cosT,
            "sinT": sinT_signed,
            "maskd": maskd,
            "ident": ident,
            "ones": ones,
        })
    return in_maps


def kernel(hidden_states, wqkv, wo, attention_mask, position_ids):
    global _EXEC_TIME_NS
    in_maps = _host_inputs(hidden_states, wqkv, wo, attention_mask,
                           position_ids)
    nc = build_nc()
    trace = os.environ.get("TRN_KERNEL_TRACE", "0") == "1"
    res = run_bass_kernel_spmd(nc, in_maps, core_ids=list(range(NCORES)),
                               trace=trace)
    _EXEC_TIME_NS = res.exec_time_ns
    out = np.zeros((S, H), dtype=np.float32)
    for c in range(NCORES):
        out += res.results[c]["out"]
    return out.reshape(1, S, H).astype(np.float32)


# revision 9
# speedup vs baseline: 1.0991x; 1.0427x over previous
"""InternLM2 attention layer on 8 Trainium2 NeuronCores.

Tensor-parallel over kv-head groups: core c gets wqkv rows [768c, 768c+768)
(4 q heads + k + v of kv-group c) and wo columns [512c, 512c+512).
Each core computes its 4 heads' attention and a partial output projection;
the host sums the 8 partials (the TP all-reduce) as the unshard step.

Device layout choices:
  - QKV computed transposed ([feat, seq]) so Q.K^T needs no transposes.
  - Scores computed pre-transposed S^T[sk, sq] so the PV matmul consumes
    exp(S^T) directly; softmax denominator via an all-ones matmul.
  - float32r matmuls (fp32 bits, ~tf32 precision, 4x fp32 throughput).
  - Causal mask baked in: fully-masked score tiles are skipped, diagonal
    tiles multiplied by a 0/1 mask after exp.
"""
import os
import math
import numpy as np

import concourse.bacc as bacc
import concourse.tile as tile
from concourse import mybir
from concourse.bass_utils import run_bass_kernel_spmd

S = 2048          # sequence length
H = 4096          # hidden size
HD = 128          # head dim
G = 4             # q heads per kv head (= per core)
FEAT = (G + 2) * HD   # 768 wqkv rows per core
NCORES = 8
CH = 512          # seq chunk (matmul moving-dim)
NCH = S // CH     # 4
KT = H // 128     # 32 contraction tiles for the projections
NSEQT = S // 128  # 16
ROPE_BASE = 1000000.0

F32 = mybir.dt.float32
F32R = mybir.dt.float32r

_EXEC_TIME_NS = None


def build_nc():
    nc = bacc.Bacc("TRN2", target_bir_lowering=False, debug=False)

    xT = nc.declare_dram_parameter("xT", [H, S], F32, isOutput=False)
    wqkvT = nc.declare_dram_parameter("wqkvT", [H, FEAT], F32, isOutput=False)
    woT = nc.declare_dram_parameter("woT", [G * HD, H], F32, isOutput=False)
    cosT = nc.declare_dram_parameter("cosT", [HD, S], F32, isOutput=False)
    sinT = nc.declare_dram_parameter("sinT", [HD, S], F32, isOutput=False)
    maskd = nc.declare_dram_parameter("maskd", [G, 128, CH], F32, isOutput=False)
    ident = nc.declare_dram_parameter("ident", [128, 128], F32, isOutput=False)
    ones = nc.declare_dram_parameter("ones", [128, 1], F32, isOutput=False)
    out = nc.declare_dram_parameter("out", [S, H], F32, isOutput=True)

    ExpF = mybir.ActivationFunctionType.Exp

    with tile.TileContext(nc) as tc:
        with tc.tile_pool(name="pers", bufs=1) as pers:
            # persistent across phases
            qT = [pers.tile([128, S], F32R, tag=f"qT{h}", name=f"qT{h}")
                  for h in range(G)]
            kT_sb = pers.tile([128, S], F32R, tag="kT")
            v_sb = pers.tile([128, NSEQT, HD], F32R, tag="v")  # [sk, t, hd]
            ident_sb = pers.tile([128, 128], F32, tag="ident")
            ones_sb = pers.tile([128, 1], F32R, tag="ones")
            nc.sync.dma_start(out=ident_sb, in_=ident[:])
            nc.sync.dma_start(out=ones_sb, in_=ones[:].bitcast(F32R))

            # ---------------- Phase 1: QKV projection + RoPE ----------------
            with tc.tile_pool(name="p1", bufs=1) as p1, \
                 tc.tile_pool(name="xt", bufs=6) as xpool, \
                 tc.tile_pool(name="rope", bufs=3) as rpool, \
                 tc.tile_pool(name="psq", bufs=1, space="PSUM") as psq, \
                 tc.tile_pool(name="psvt", bufs=2, space="PSUM") as psvt:
                w_sb = p1.tile([128, KT, FEAT], F32R, tag="w")
                wqkvT_r = wqkvT[:].bitcast(F32R).rearrange(
                    "(kt p) f -> kt p f", p=128)
                cos_sb = p1.tile([128, S], F32, tag="cos")
                sin_sb = p1.tile([128, S], F32, tag="sin")
                vT_sb = p1.tile([128, S], F32, tag="vT")  # v as [hd, seq]

                xT_r = xT[:].bitcast(F32R)
                for ch in range(NCH):
                    cc = slice(ch * CH, (ch + 1) * CH)
                    ps = [psq.tile([128, CH], F32, tag=f"ps{m}", name=f"ps{m}")
                          for m in range(6)]
                    for k in range(KT):
                        if ch == 0:
                            # interleave weight loads with x tiles so the
                            # first matmuls aren't behind the full 12.6MB
                            nc.sync.dma_start(out=w_sb[:, k, :],
                                              in_=wqkvT_r[k])
                        xt = xpool.tile([128, CH], F32R, tag="x")
                        nc.sync.dma_start(
                            out=xt, in_=xT_r[k * 128:(k + 1) * 128, cc])
                        for m in range(6):
                            nc.tensor.matmul(
                                ps[m],
                                lhsT=w_sb[:, k, m * 128:(m + 1) * 128],
                                rhs=xt,
                                start=(k == 0),
                                stop=(k == KT - 1),
                            )
                    if ch == 0:
                        nc.sync.dma_start(out=cos_sb, in_=cosT[:])
                        nc.sync.dma_start(out=sin_sb, in_=sinT[:])
                    # drain: RoPE for q heads (m=0..3) and k (m=4)
                    for m in range(5):
                        dst = qT[m] if m < G else kT_sb
                        qp = rpool.tile([128, CH], F32, tag="qp")
                        if m % 2 == 0:
                            nc.scalar.copy(qp, ps[m])
                        else:
                            nc.vector.tensor_copy(qp, ps[m])
                        sh = rpool.tile([128, CH], F32, tag="sh")
                        nc.sync.dma_start(out=sh[0:64, :], in_=qp[64:128, :])
                        nc.sync.dma_start(out=sh[64:128, :], in_=qp[0:64, :])
                        nc.vector.tensor_mul(dst[:, cc], qp, cos_sb[:, cc])
                        nc.vector.tensor_mul(sh, sh, sin_sb[:, cc])
                        nc.vector.tensor_add(dst[:, cc], dst[:, cc], sh)
                    # v: plain copy (no rope), then transpose this chunk's
                    # v tiles to [sk, hd] while later chunks still project
                    nc.scalar.copy(vT_sb[:, cc], ps[5])
                    for t in range(G * ch, G * ch + G):
                        pst = psvt.tile([128, 128], F32, tag="vt")
                        nc.tensor.transpose(
                            pst, vT_sb[:, t * 128:(t + 1) * 128], ident_sb)
                        nc.any.tensor_copy(v_sb[:, t, :], pst)

            # ---------------- Phase 2+3: attention + out-proj ----------------
            with tc.tile_pool(name="p2", bufs=1) as p2, \
                 tc.tile_pool(name="epool", bufs=8) as epool, \
                 tc.tile_pool(name="spool", bufs=4) as spool, \
                 tc.tile_pool(name="opool", bufs=4) as opool, \
                 tc.tile_pool(name="psst", bufs=2, space="PSUM") as psst, \
                 tc.tile_pool(name="pspv", bufs=2, space="PSUM") as pspv, \
                 tc.tile_pool(name="psden", bufs=2, space="PSUM") as psden, \
                 tc.tile_pool(name="pso", bufs=2, space="PSUM") as pso:
                wo_sb = p2.tile([128, G, H], F32R, tag="wo")
                woT_r = woT[:].bitcast(F32R).rearrange("(g p) o -> g p o", p=128)
                for g in range(G):
                    nc.sync.dma_start(out=wo_sb[:, g, :], in_=woT_r[g])
                masks = p2.tile([128, G, CH], F32, tag="masks")
                nc.sync.dma_start(
                    out=masks, in_=maskd[:].rearrange("r p c -> p r c"))
                attnT = [p2.tile([128, S], F32R, tag=f"attnT{h}", name=f"attnT{h}")
                         for h in range(G)]

                for ch in range(NCH):
                    for h in range(G):
                        cc = slice(ch * CH, (ch + 1) * CH)
                        nsk = G * ch + G
                        pv = pspv.tile([128, CH], F32, tag="pv")
                        den = psden.tile([1, CH], F32, tag="den")
                        for t in range(nsk):
                            st = psst.tile([128, CH], F32, tag="st")
                            nc.tensor.matmul(
                                st,
                                lhsT=kT_sb[:, t * 128:(t + 1) * 128],
                                rhs=qT[h][:, cc],
                                start=True, stop=True)
                            E = epool.tile([128, CH], F32R, tag="E")
                            nc.scalar.activation(E, st, ExpF)
                            if t >= G * ch:  # diagonal tile: causal 0/1 mask
                                nc.vector.tensor_mul(
                                    E, E, masks[:, t - G * ch, :])
                            nc.tensor.matmul(
                                pv, lhsT=v_sb[:, t, :], rhs=E,
                                start=(t == 0), stop=(t == nsk - 1))
                            nc.tensor.matmul(
                                den, lhsT=ones_sb, rhs=E,
                                start=(t == 0), stop=(t == nsk - 1))
                        recip = spool.tile([1, CH], F32, tag="recip")
                        nc.vector.reciprocal(recip, den[0:1, :])
                        bc = spool.tile([128, CH], F32, tag="bc")
                        nc.gpsimd.partition_broadcast(bc, recip[0:1, :])
                        nc.vector.tensor_mul(attnT[h][:, cc], pv, bc)

                # out projection: out[sq, :] partial
                for mt in range(NSEQT):
                    sq = slice(mt * 128, (mt + 1) * 128)
                    for n in range(H // CH):
                        po = pso.tile([128, CH], F32, tag="po")
                        for g in range(G):
                            nc.tensor.matmul(
                                po,
                                lhsT=attnT[g][:, sq],
                                rhs=wo_sb[:, g, n * CH:(n + 1) * CH],
                                start=(g == 0), stop=(g == G - 1))
                        ot = opool.tile([128, CH], F32, tag="ot")
                        if n % 2 == 0:
                            nc.vector.tensor_copy(ot, po)
                        else:
                            nc.scalar.copy(ot, po)
                        nc.sync.dma_start(out=out[:][sq, n * CH:(n + 1) * CH],
                                          in_=ot)

    nc.finalize()
    return nc


def _host_inputs(hidden_states, wqkv, wo, attention_mask, position_ids):
    x = np.asarray(hidden_states, dtype=np.float32)[0]          # [S, H]
    wqkv = np.asarray(wqkv, dtype=np.float32)                   # [6144, H]
    wo = np.asarray(wo, dtype=np.float32)                       # [H, H]
    pos = np.asarray(position_ids)[0].astype(np.int64)          # [S]

    xT = np.ascontiguousarray(x.T)                              # [H, S]

    inv = 1.0 / (ROPE_BASE ** (np.arange(0, HD, 2, dtype=np.float32) / HD))
    freqs = np.outer(pos.astype(np.float32), inv)               # [S, 64]
    emb = np.concatenate([freqs, freqs], axis=-1)               # [S, 128]
    cosT = np.ascontiguousarray(np.cos(emb).T.astype(np.float32))   # [128, S]
    sinT = np.sin(emb).T.astype(np.float32)
    sinT_signed = sinT.copy()
    sinT_signed[:64] = -sinT_signed[:64]
    sinT_signed = np.ascontiguousarray(sinT_signed)

    # diagonal-tile causal 0/1 masks, [G, 128, CH]
    r_off = (np.arange(G) * 128)[:, None, None]
    i_idx = np.arange(128)[None, :, None]
    c_idx = np.arange(CH)[None, None, :]
    maskd = (c_idx >= r_off + i_idx).astype(np.float32)

    ident = np.eye(128, dtype=np.float32)
    ones = np.ones((128, 1), dtype=np.float32)

    scale = np.float32(1.0 / math.sqrt(HD))
    in_maps = []
    for c in range(NCORES):
        wq_c = wqkv[FEAT * c:FEAT * (c + 1)].copy()             # [768, H]
        wq_c[:G * HD] *= scale                                  # fold softmax scale
        wqkvT_c = np.ascontiguousarray(wq_c.T)                  # [H, 768]
        woT_c = np.ascontiguousarray(wo[:, G * HD * c:G * HD * (c + 1)].T)
        in_maps.append({
            "xT": xT,
            "wqkvT": wqkvT_c,
            "woT": woT_c,
            "cosT": cosT,
            "sinT": sinT_signed,
            "maskd": maskd,
            "ident": ident,
            "ones": ones,
        })
    return in_maps


def kernel(hidden_states, wqkv, wo, attention_mask, position_ids):
    global _EXEC_TIME_NS
    in_maps = _host_inputs(hidden_states, wqkv, wo, attention_mask,
                           position_ids)
    nc = build_nc()
    trace = os.environ.get("TRN_KERNEL_TRACE", "0") == "1"
    res = run_bass_kernel_spmd(nc, in_maps, core_ids=list(range(NCORES)),
                               trace=trace)
    _EXEC_TIME_NS = res.exec_time_ns
    out = np.zeros((S, H), dtype=np.float32)
    for c in range(NCORES):
        out += res.results[c]["out"]
    return out.reshape(1, S, H).astype(np.float32)
